# revision 40
# baseline (speedup 1.0000x reference)
"""ChaosAttention Trainium2 kernel.

Problem: B=2, L=2048, D=1024, H=16 heads (hd=64), chaos-gated attention.

Sharding (8 NeuronCores): data-parallel over B (2) x tensor-parallel over
head groups (4 groups of 4 heads). Core c handles batch b=c//4, head group
g=c%4 (global heads 4g..4g+3). q/k/v/chaos projections are column-sharded,
out_proj row-sharded; per-core partial outputs are summed on host.

Device structure (single fused pipeline; the Act engine's exp stream is
the critical resource, so projections are interleaved as "feeder" PE work
scheduled by deadline between attention steps):

  startup: project k/v for key-quarter 0 + q for query-chunk 0
  for qc in 0..3:                      # 512-query chunks
    for wave in (h0,h1), (h2,h3):     # 2 heads per wave (PSUM budget)
      for g in 0..7:                  # pairs of 128-key tiles
        S^T = K'_h^T-contract-Q'_h    # fused chaos via [k;gate*k]x[q;cq]
        P   = exp(S^T/8) -> fp8e4
        out += [v_hi|1] @ P, out += [v_lo|0] @ P   # fp8 DoubleRow (K=256)
      normalize wave heads            # ones-col denominator, recip bcast
    feeders: k/v quarters 1-3 (during qc0), q chunk qc+1, out_proj qc-1
  tail: out_proj(qc3)

v_lo = v - fp8(v) is an error-feedback residual: fp8 alone loses ~3.6%
relative on the attention output (softmax averaging shrinks signal and
quantization noise equally); the residual pass recovers it at half the
PE cost of a bf16 AV matmul. exp output stays fp8 (~1% contribution).

Host does only O(B*L*D) glue: x transpose, Lorenz chaos field, gate,
chaos-feature slices, weight slicing, final 4-way partial sum (+ bo +
bv@Wo which are exact row-parallel bias corrections).
"""

import sys

if "/opt/trn_rl_repo" not in sys.path:
    sys.path.insert(0, "/opt/trn_rl_repo")

import ml_dtypes
import numpy as np

import concourse.bacc as bacc
import concourse.mybir as mybir
import concourse.tile as tile
from concourse.bass_utils import run_bass_kernel_spmd

# Problem constants (hardcoded per contract)
B, L, D = 2, 2048, 1024
H, HD = 16, 64
H4 = 4                  # heads per core
DG = H4 * HD            # 256 = head-group width
KB = D // 128           # 8 contraction blocks
CHAOS_STRENGTH = np.float32(0.1)
SIGMA, RHO, BETA, DT = 10.0, 28.0, 8.0 / 3.0, 0.01
N_LORENZ_STEPS = 10
SCALE = 1.0 / 8.0       # 1/sqrt(HD)

F32 = mybir.dt.float32
F32R = mybir.dt.float32r
BF16 = mybir.dt.bfloat16
FP8 = mybir.dt.float8e4
DR = mybir.MatmulPerfMode.DoubleRow

_CACHED = {}


def _build_nc():
    nc = bacc.Bacc()

    xT = nc.dram_tensor("xT", [D, L], BF16, kind="ExternalInput")
    wq = nc.dram_tensor("wq", [D, DG], BF16, kind="ExternalInput")
    wk = nc.dram_tensor("wk", [D, DG], BF16, kind="ExternalInput")
    wv = nc.dram_tensor("wv", [D, DG], BF16, kind="ExternalInput")
    wo = nc.dram_tensor("wo", [DG, D], F32R, kind="ExternalInput")
    cqt = nc.dram_tensor("cqt", [DG, L], BF16, kind="ExternalInput")
    gateB = nc.dram_tensor("gateB", [128, L], BF16, kind="ExternalInput")
    bqv = nc.dram_tensor("bqv", [128, H4], F32, kind="ExternalInput")
    bkv = nc.dram_tensor("bkv", [128, H4], F32, kind="ExternalInput")
    onesd = nc.dram_tensor("onesd", [128, HD], F32R, kind="ExternalInput")
    onespad = nc.dram_tensor("onespad", [128, 8192], BF16, kind="ExternalInput")
    out = nc.dram_tensor("out", [L, D], F32, kind="ExternalOutput")

    Exp = mybir.ActivationFunctionType.Exp
    MUL = mybir.AluOpType.mult
    SUB = mybir.AluOpType.subtract

    with tile.TileContext(nc) as tc:
        with (
            tc.tile_pool(name="persist", bufs=1) as pp,
            tc.tile_pool(name="scps", bufs=2, space="PSUM") as scps,
            tc.tile_pool(name="avps", bufs=2, space="PSUM") as avps,
            tc.tile_pool(name="mmps", bufs=2, space="PSUM") as mmps,
            tc.tile_pool(name="expp", bufs=3) as expp,
            tc.tile_pool(name="rcpp", bufs=2) as rcpp,
            tc.tile_pool(name="fsb", bufs=3) as fsbp,
        ):
            qp = [pp.tile([128, L], BF16, tag=f"qp{h}", name=f"qp{h}") for h in range(H4)]
            kp = [pp.tile([128, L], BF16, tag=f"kp{h}", name=f"kp{h}") for h in range(H4)]
            # v2[p, pair, head, j, c]: c = 64 v-dims | ones col | zero pad
            v2 = pp.tile([128, 8, H4, 2, 128], BF16, tag="v2")
            gb = pp.tile([128, L], BF16, tag="gb")
            ones_t = pp.tile([128, HD], F32R, tag="ones")
            bq_sb = pp.tile([128, H4], F32, tag="bq")
            bk_sb = pp.tile([128, H4], F32, tag="bk")
            pre = pp.tile([1, 2], F32, tag="pre")
            xt = pp.tile([128, KB, L], BF16, tag="xt")
            wq_sb = pp.tile([128, KB, DG], BF16, tag="wq")
            wk_sb = pp.tile([128, KB, DG], BF16, tag="wk")
            wv_sb = pp.tile([128, KB, DG], BF16, tag="wv")
            wo_sb = pp.tile([128, 2, D], F32R, tag="wo")
            otm = [pp.tile([128, L], F32R, tag=f"otm{p}", name=f"otm{p}")
                   for p in range(2)]
            ots = [pp.tile([64, L], F32R, tag=f"ots{p}", name=f"ots{p}")
                   for p in range(2)]

            # ---- input DMA, deadline order (SP HWDGE queue). Each is ONE
            # multi-descriptor dma_start: HWDGE issue is ~650ns per
            # instruction, so per-kb DMAs would serialize the startup ----
            def dma_w(dst, src):
                nc.sync.dma_start(
                    out=dst[:], in_=src.rearrange("(kb p) c -> p kb c", p=128)
                )

            def dma_xq(lc):
                cs = slice(lc * 512, lc * 512 + 512)
                nc.sync.dma_start(
                    out=xt[:, :, cs],
                    in_=xT[:, cs].rearrange("(kb p) c -> p kb c", p=128),
                )

            # single SP HWDGE channel, strict deadline order (the DMA
            # engines serialize transfers, so issue order = landing order)
            nc.sync.dma_start(out=bk_sb[:], in_=bkv[:])  # tiny; unblocks pre
            dma_w(wk_sb, wk)
            dma_xq(0)
            dma_w(wq_sb, wq)
            nc.sync.dma_start(out=gb[:], in_=gateB[:])
            # chaos features into the q tiles (odd heads use flipped layout)
            for h in range(H4):
                dst = (qp[h][64:128, :] if h % 2 == 0 else qp[h][0:64, :])
                nc.sync.dma_start(out=dst, in_=cqt[64 * h:64 * h + 64, :])
            nc.sync.dma_start(out=bq_sb[:], in_=bqv[:])
            dma_w(wv_sb, wv)
            # whole-v2 image (zeros + ones column): a contiguous transfer;
            # writing just cols 64:66 would be 16K two-byte descriptors
            nc.sync.dma_start(
                out=v2[:],
                in_=onespad.rearrange(
                    "p (g h j c) -> p g h j c", h=H4, j=2, c=128
                ),
            )
            nc.sync.dma_start(out=ones_t[:], in_=onesd[:, 0:HD])
            dma_xq(1)
            dma_xq(2)
            dma_xq(3)
            dma_w(wo_sb, wo)

            # preload the Exp table off the critical path (first activation
            # triggers the table load)
            nc.scalar.activation(pre[:], bk_sb[0:1, 0:2], Exp)

            # ---- feeder building blocks (each issues PE work + evac) ----
            def kproj(lc, p, dge=None):
                # kT for one key quarter, head pair p; gated halves via
                # cross-partition DMA copy then DVE multiply
                dge = dge or nc.gpsimd
                cs = slice(lc * 512, lc * 512 + 512)
                he, ho = 2 * p, 2 * p + 1
                ps = mmps.tile([128, 512], F32, tag="mm")
                for kb in range(KB):
                    nc.tensor.matmul(
                        ps[:],
                        wk_sb[:, kb, 128 * p:128 * p + 128],
                        xt[:, kb, cs],
                        start=(kb == 0),
                        stop=(kb == KB - 1),
                    )
                nc.vector.tensor_scalar_add(
                    kp[he][0:64, cs], ps[0:64, :], bk_sb[0:64, he:he + 1],
                )
                nc.vector.tensor_scalar_add(
                    kp[ho][64:128, cs], ps[64:128, :],
                    bk_sb[64:128, ho:ho + 1],
                )
                dge.dma_start(
                    out=kp[he][64:128, cs], in_=kp[he][0:64, cs]
                )
                nc.vector.tensor_tensor(
                    out=kp[he][64:128, cs], in0=kp[he][64:128, cs],
                    in1=gb[64:128, cs], op=MUL,
                )
                dge.dma_start(
                    out=kp[ho][0:64, cs], in_=kp[ho][64:128, cs]
                )
                nc.vector.tensor_tensor(
                    out=kp[ho][0:64, cs], in0=kp[ho][0:64, cs],
                    in1=gb[0:64, cs], op=MUL,
                )

            def qproj(lc, p):
                cs = slice(lc * 512, lc * 512 + 512)
                he, ho = 2 * p, 2 * p + 1
                ps = mmps.tile([128, 512], F32, tag="mm")
                for kb in range(KB):
                    nc.tensor.matmul(
                        ps[:],
                        wq_sb[:, kb, 128 * p:128 * p + 128],
                        xt[:, kb, cs],
                        start=(kb == 0),
                        stop=(kb == KB - 1),
                    )
                nc.vector.tensor_scalar_add(
                    qp[he][0:64, cs], ps[0:64, :], bq_sb[0:64, he:he + 1],
                )
                nc.vector.tensor_scalar_add(
                    qp[ho][64:128, cs], ps[64:128, :],
                    bq_sb[64:128, ho:ho + 1],
                )

            def vproj(lt):
                # v for one 128-key l-tile -> fp8 hi + error-feedback lo
                ps = mmps.tile([128, 512], F32, tag="mm")
                for kb in range(KB):
                    nc.tensor.matmul(
                        ps[:, 0:DG],
                        xt[:, kb, lt * 128:lt * 128 + 128],
                        wv_sb[:, kb, :],
                        start=(kb == 0),
                        stop=(kb == KB - 1),
                    )
                nc.vector.tensor_copy(
                    v2[:, lt // 2, :, lt % 2, 0:64],
                    ps[:, 0:DG].rearrange("p (h d) -> p h d", d=64),
                )

            def outproj(lt, fs):
                for nch in range(2):
                    fp = mmps.tile([128, 512], F32, tag="mm")
                    for p in range(2):
                        nc.tensor.matmul(
                            fp[:],
                            otm[p][:, lt * 128:lt * 128 + 128],
                            wo_sb[:, p, nch * 512:nch * 512 + 512],
                            start=(p == 0),
                            stop=(p == 1),
                        )
                    nc.vector.tensor_copy(
                        fs[:, nch * 512:nch * 512 + 512], fp[:]
                    )
                # SP HWDGE is idle once inputs have landed
                nc.sync.dma_start(
                    out=out[lt * 128:lt * 128 + 128, :], in_=fs[:]
                )

            # ---- startup projections: just enough for wave 0 g0 (the
            # gating copy rides the Act HWDGE, idle until the first exp) ----
            kproj(0, 0, dge=nc.scalar)
            qproj(0, 0)
            vproj(0)
            vproj(1)

            # ---- fused attention + feeders ----
            def scores_step(h, g, qs):
                sps = scps.tile([128, 1024], F32, tag="sc")
                for j in range(2):
                    kt = 2 * g + j
                    nc.tensor.matmul(
                        sps[:, j * 512:j * 512 + 512],
                        kp[h][:, kt * 128:kt * 128 + 128],
                        qp[h][:, qs],
                        start=True,
                        stop=True,
                    )
                et = expp.tile([128, 1024], BF16, tag="et")
                nc.scalar.activation(et[:], sps[:], Exp, scale=SCALE)
                return et.rearrange("p (j q) -> p j q", j=2)

            def av_step(h, g, aps, e2):
                for j in range(2):
                    nc.tensor.matmul(
                        aps[0:66, :],
                        v2[:, g, h, j, 0:66],
                        e2[:, j, :],
                        start=(g == 0 and j == 0),
                        stop=(g == 7 and j == 1),
                        skip_group_check=True,
                    )

            def norm(h, qs, aps):
                rcp = rcpp.tile([128, 512], F32R, tag="rcp")
                with nc.allow_low_precision(
                    "f32r reciprocal feeds a K=1 broadcast matmul; "
                    "f32r rounding is ~1e-4 relative"
                ):
                    nc.vector.reciprocal(rcp[64:65, :], aps[64:65, :])
                psb = mmps.tile([128, 512], F32, tag="mm")
                nc.tensor.matmul(
                    psb[0:64, :],
                    ones_t[64:65, 0:64],
                    rcp[64:65, :],
                    start=True,
                    stop=True,
                )
                rcb = rcpp.tile([128, 512], F32, tag="rcb")
                nc.vector.tensor_copy(rcb[0:64, :], psb[0:64, :])
                dst = (otm[h // 2][0:64, qs] if h % 2 == 0
                       else ots[h // 2][:, qs])
                nc.vector.scalar_tensor_tensor(
                    dst,
                    in0=aps[0:64, :],
                    scalar=1.0,
                    in1=rcb[0:64, :],
                    op0=MUL,
                    op1=MUL,
                )
                if h % 2 == 1:
                    nc.sync.dma_start(
                        out=otm[h // 2][64:128, qs],
                        in_=ots[h // 2][:, qs],
                    )

            # feeders[qc][wave] = {"pre": [...], g: [...]} thunks issued
            # between the scores and AV matmuls of step g (PE is in-order,
            # so issue position == deadline position)
            def mkout(lt):
                def f():
                    fs = fsbp.tile([128, D], F32, tag="fs")
                    outproj(lt, fs)
                return f

            feeders = {qc: {w: {g: [] for g in range(8)} for w in range(2)}
                       for qc in range(4)}
            fq0 = feeders[0][0]
            fq0[0] += [lambda: vproj(2), lambda: vproj(3)]
            fq0[1] += [lambda: kproj(1, 0), lambda: vproj(4), lambda: vproj(5)]
            fq0[2] += [lambda: vproj(6), lambda: vproj(7)]
            fq0[3] += [lambda: kproj(2, 0), lambda: vproj(8), lambda: vproj(9)]
            fq0[4] += [lambda: vproj(10), lambda: vproj(11)]
            fq0[5] += [lambda: kproj(3, 0), lambda: vproj(12), lambda: vproj(13)]
            fq0[6] += [lambda: vproj(14), lambda: vproj(15), lambda: kproj(0, 1)]
            fq0[7] += [lambda: qproj(0, 1), lambda: kproj(1, 1)]
            fq1 = feeders[0][1]
            fq1[1] += [lambda: kproj(2, 1)]
            fq1[3] += [lambda: kproj(3, 1)]
            fq1[5] += [lambda: qproj(1, 0)]
            fq1[6] += [lambda: qproj(1, 1)]
            for qc in range(1, 4):
                # out_proj for the previous chunk + next q projection
                for i, g in enumerate((1, 3, 5, 7)):
                    feeders[qc][0][g].append(mkout(4 * (qc - 1) + i))
                if qc < 3:
                    feeders[qc][1][2].append(lambda lc=qc + 1: qproj(lc, 0))
                    feeders[qc][1][4].append(lambda lc=qc + 1: qproj(lc, 1))

            # norms for a wave are deferred into the NEXT wave's g0 feeder
            # slot (between its scores and AV matmuls) so the exp stream
            # never waits on the normalization chain
            pending = []
            for qc in range(4):
                qs = slice(qc * 512, qc * 512 + 512)
                for w in range(2):
                    hs = (2 * w, 2 * w + 1)
                    aps = {h: avps.tile([128, 512], F32, tag="av",
                                        name=f"aps{qc}_{h}")
                           for h in hs}
                    for g in range(8):
                        ets = {h: scores_step(h, g, qs) for h in hs}
                        if g == 0:
                            for thunk in pending:
                                thunk()
                            pending = []
                        for thunk in feeders[qc][w][g]:
                            thunk()
                        for h in hs:
                            av_step(h, g, aps[h], ets[h])
                    pending = [
                        (lambda h=h, qs=qs, t=aps[h]: norm(h, qs, t))
                        for h in hs
                    ]

            # tail: last wave's norms + out_proj for the last chunk
            for thunk in pending:
                thunk()
            for lt in range(12, 16):
                fs = fsbp.tile([128, D], F32, tag="fs")
                outproj(lt, fs)

    nc.finalize()
    return nc


def _chaos_field(ci):
    """Replicates reference _chaos_field in float32 numpy."""
    xv = ci[..., 0].astype(np.float32)
    yv = ci[..., 1].astype(np.float32)
    zv = ci[..., 2].astype(np.float32)
    sigma = np.float32(SIGMA)
    rho = np.float32(RHO)
    beta = np.float32(BETA)
    dt = np.float32(DT)
    acc = np.zeros(ci.shape, dtype=np.float32)
    for _ in range(N_LORENZ_STEPS):
        dx = sigma * (yv - xv)
        dy = xv * (rho - zv) - yv
        dz = xv * yv - beta * zv
        xv = xv + dt * dx
        yv = yv + dt * dy
        zv = zv + dt * dz
        acc = acc + np.stack([xv, yv, zv], axis=-1)
    return acc / np.float32(N_LORENZ_STEPS)


def _make_onespad():
    # full v2 image [p, (g h j c=128)]: zeros, with c=64 = 1 (ones column)
    arr = np.zeros((128, 8192), dtype=np.float32)
    arr[:, np.arange(8192) % 128 == 64] = 1.0
    return arr.astype(ml_dtypes.bfloat16)


def _prepare_in_maps(x, chaos_init, Wq, bq, Wk, bk, Wv, Wc, bc, Wg, bg):
    cf = _chaos_field(np.asarray(chaos_init, dtype=np.float32))  # [B,L,3]
    # gate = sigmoid(cf @ (Wc@Wg) + bc@Wg + bg), folded over the tiny K=3
    cfeat = cf @ Wc + bc                                        # [B,L,D]
    gate_logit = cfeat @ Wg + bg                                # [B,L,1]
    gate = (1.0 / (1.0 + np.exp(-gate_logit[..., 0]))).astype(np.float32)
    cq = (CHAOS_STRENGTH * cfeat).astype(np.float32)            # [B,L,D]

    in_maps = []
    for c in range(8):
        b, g = c // 4, c % 4
        gsl = slice(DG * g, DG * g + DG)
        bq_g = bq[gsl].astype(np.float32)
        bqv_m = np.empty((128, H4), dtype=np.float32)
        for h in range(H4):
            bqv_m[0:64, h] = bq_g[64 * h:64 * h + 64]
            bqv_m[64:128, h] = bq_g[64 * h:64 * h + 64]
        bk_g = bk[gsl].astype(np.float32)
        bkv = np.empty((128, H4), dtype=np.float32)
        for h in range(H4):
            bkv[0:64, h] = bk_g[64 * h:64 * h + 64]
            bkv[64:128, h] = bk_g[64 * h:64 * h + 64]
        bf = ml_dtypes.bfloat16
        in_maps.append({
            "xT": np.ascontiguousarray(x[b].T.astype(bf)),
            "wq": np.ascontiguousarray(Wq[:, gsl].astype(bf)),
            "wk": np.ascontiguousarray(Wk[:, gsl].astype(bf)),
            "wv": np.ascontiguousarray(Wv[:, gsl].astype(bf)),
            "wo": None,  # filled below (needs Wo)
            "cqt": np.ascontiguousarray(cq[b][:, gsl].T.astype(bf)),
            "gateB": np.ascontiguousarray(
                np.broadcast_to(gate[b], (128, L)).astype(bf)
            ),
            "bqv": bqv_m,
            "bkv": bkv,
            "onesd": np.ones((128, HD), dtype=np.float32),
            "onespad": _make_onespad(),
        })
    return in_maps


def kernel(x, mask, chaos_init, Wq, bq, Wk, bk, Wv, bv, Wo, bo, Wc, bc, Wg, bg):
    x = np.asarray(x, dtype=np.float32)
    Wq, Wk, Wv, Wo = (np.asarray(a, dtype=np.float32) for a in (Wq, Wk, Wv, Wo))
    Wc, Wg = np.asarray(Wc, np.float32), np.asarray(Wg, np.float32)
    bq, bk, bv, bo = (np.asarray(a, np.float32) for a in (bq, bk, bv, bo))
    bc, bg = np.asarray(bc, np.float32), np.asarray(bg, np.float32)

    if "nc" not in _CACHED:
        _CACHED["nc"] = _build_nc()
    nc = _CACHED["nc"]

    in_maps = _prepare_in_maps(x, chaos_init, Wq, bq, Wk, bk, Wv, Wc, bc, Wg, bg)
    for c in range(8):
        g = c % 4
        in_maps[c]["wo"] = np.ascontiguousarray(Wo[DG * g:DG * g + DG, :])

    res = run_bass_kernel_spmd(nc, in_maps, list(range(8)))

    # host unshard: sum row-parallel partials per batch, add bias terms
    bias_row = (bv @ Wo + bo).astype(np.float32)                # [D]
    out = np.empty((B, L, D), dtype=np.float32)
    for b in range(B):
        acc = res.results[4 * b + 0]["out"].astype(np.float32).copy()
        for g in range(1, 4):
            acc += res.results[4 * b + g]["out"]
        out[b] = acc + bias_row
    return out
